# revision 11
# baseline (speedup 1.0000x reference)
"""GAT 2-layer kernel for Trainium2, 8 NeuronCores (SPMD, dst-sharded).

Strategy (v4):
  - Destination-node sharding: core c owns nodes [c*6250,(c+1)*6250); non-self
    edges bucketed into per-128-dst-node "slots", padded to 128-edge chunks.
  - Stage A (replicated): per 4x128-node group one 256KB read of xT, four
    matmuls [x@W1 (192) | alpha_src (3)] -> bf16 gather table G1 (512B rows:
    192 bf16 xw + 3 f32 alpha_src bit-packed + pad), one 256KB batched write.
    G1 split into two <=32768-row tensors (dma_gather int16 index limit).
  - One-hot matrices S8 (e->d) and its transpose ST8 are STATIC functions of
    the edge structure: precomputed host-side as bf16, streamed from HBM per
    op group (replaces on-device DVE compares + PE broadcast matmuls).
  - alpha_dst per slot computed locally from a per-core xTself input slice
    (PE matmul vs rhs1 ad-columns) -- no AD tables, no indirect gathers.
  - Self-loop edges excluded from the gather stream; per-slot self
    contribution (exp(lrelu(as+ad)) * xw_self) computed from xTself / the
    layer-1 epilogue stash and DVE-added into the slot PSUM at epilogue.
  - Edge phase per layer: per <=8-chunk group one dma_gather (1024 rows/op)
    pulls source rows; adp = ST8 x adb expands alpha_dst edge-wise; per-chunk
    segment matmul S8^T x F8 accumulates exp-weighted features + softmax
    denominators in per-slot PSUM (normalization pulled out of the sum).
  - Per-slot epilogue: h = relu((sum+self)/(denom+eps) + bias1); PE-transpose
    h, emit G2 rows [h@W2 (64) bf16 | as2 f32], stash ad2/g2 rows in SBUF for
    layer-2 self/alpha_dst; single AllGather of G2; layer 2 repeats the edge
    phase (1 head) against G2F views.
"""
import sys

sys.path.insert(0, "/opt/trn_rl_repo")
import numpy as np
import ml_dtypes

N = 50000
D = 128
HID = 64
H = 3
F1 = 192
F2 = 64
NCORES = 8
NPC = N // NCORES          # 6250 nodes per core
P = 128
NBLK = (NPC + P - 1) // P  # 49 slots per core
NT = (N + P - 1) // P      # 391 stage-A node tiles
NROW1 = NT * P             # 50048 G1 rows
HALF = 32768               # dma_gather int16 index limit
G1W = 256                  # bf16 cols: xw(192) | as f32 x3 (bf16 192:198) | pad
G2W = 128                  # bf16 cols: xw2(64) | as2 f32 (bf16 64:66) | pad
NROWC = NBLK * P           # 6272 rows per core shard
SLOPE = 0.2
EPS = 1e-16
GRP = 8                    # max chunks per dma_gather / op group
TBATCH = 4                 # stage-A tiles per batched iteration

_compiled = {}


def _chunkize(src_key, dst):
    """Per (core, slot, table-half) chunk counts, maxed over cores."""
    core = dst // NPC
    slot = (dst % NPC) // P
    half = (src_key >= HALF).astype(np.int64)
    counts = np.zeros((NCORES, NBLK, 2), dtype=np.int64)
    np.add.at(counts, (core, slot, half), 1)
    Ka = np.ceil(counts[:, :, 0] / P).astype(np.int64).max(axis=0)
    Kb = np.ceil(counts[:, :, 1] / P).astype(np.int64).max(axis=0)
    return Ka, Kb


def _host_prep(inputs):
    x = np.asarray(inputs["x"], dtype=np.float32)
    ei = np.asarray(inputs["edge_index"])
    W1 = np.asarray(inputs["W1"], dtype=np.float32)
    as1 = np.asarray(inputs["att_src1"], dtype=np.float32)
    ad1 = np.asarray(inputs["att_dst1"], dtype=np.float32)
    b1 = np.asarray(inputs["bias1"], dtype=np.float32)
    W2 = np.asarray(inputs["W2"], dtype=np.float32)
    as2 = np.asarray(inputs["att_src2"], dtype=np.float32)
    ad2 = np.asarray(inputs["att_dst2"], dtype=np.float32)
    b2 = np.asarray(inputs["bias2"], dtype=np.float32)

    # self-loops are handled per-slot on device; only real edges here
    src = ei[0].astype(np.int64)
    dst = ei[1].astype(np.int64)
    order = np.argsort(dst, kind="stable")
    src = src[order]
    dst = dst[order]
    g2row = (src // NPC) * NROWC + (src % NPC)

    Ka1, Kb1 = _chunkize(src, dst)
    Ka2, Kb2 = _chunkize(g2row, dst)

    def build_layer(key):
        Ka, Kb = (Ka1, Kb1) if key == 1 else (Ka2, Kb2)
        skey = src if key == 1 else g2row
        NCH = int((Ka + Kb).sum())
        assert (Ka + Kb).min() >= 1, "empty slot: epilogue would be skipped"
        # chunk meta: (slot, k_in_slot, table) in processing order
        # b-table chunks first: their gathers only depend on the (smaller,
        # first-written) G1b table, overlapping the tail of stage A
        meta = []
        for s in range(NBLK):
            k = 0
            for _ in range(int(Kb[s])):
                meta.append((s, k, 1)); k += 1
            for _ in range(int(Ka[s])):
                meta.append((s, k, 0)); k += 1
        # gather ops: runs of <=GRP same-table consecutive chunks
        ops = []   # (chunk_start, n_chunks, table)
        i = 0
        while i < NCH:
            t = meta[i][2]
            j = i
            while j < NCH and j - i < GRP and meta[j][2] == t:
                j += 1
            ops.append((i, j - i, t))
            i = j
        NOPS = len(ops)

        EPAD = NCH * P
        SRCK = np.zeros((NCORES, EPAD), dtype=np.int64)
        DREL = np.full((NCORES, EPAD), 255.0, dtype=np.float32)
        for c in range(NCORES):
            base_node = c * NPC
            cb = 0
            for s in range(NBLK):
                blo = base_node + s * P
                bhi = min(blo + P, base_node + NPC)
                lo = np.searchsorted(dst, blo, side="left")
                hi = np.searchsorted(dst, bhi, side="left")
                sk = skey[lo:hi]
                dr = (dst[lo:hi] - blo).astype(np.float32)
                a_mask = sk < HALF
                for which, KK, pad in ((~a_mask, Kb[s], HALF),
                                       (a_mask, Ka[s], 0)):
                    cnt = int(which.sum())
                    pos = cb * P
                    SRCK[c, pos:pos + cnt] = sk[which]
                    # pad indices must stay valid for the table half
                    SRCK[c, pos + cnt:(cb + int(KK)) * P] = pad
                    DREL[c, pos:pos + cnt] = dr[which]
                    cb += int(KK)
        # static one-hot matrices, bf16:
        #   S8W[c, e, ch*128+d] = (DREL[ch, e] == d)   (segment-matmul lhsT)
        #   ST8W[c, d, ch*128+e] = same, transposed    (alpha-dst-expand lhsT)
        drel_ch = DREL.reshape(NCORES, NCH, P)
        oh = (drel_ch[:, :, :, None] ==
              np.arange(P, dtype=np.float32)[None, None, None, :])
        S8W = np.ascontiguousarray(
            oh.transpose(0, 2, 1, 3).reshape(NCORES, P, NCH * P)
        ).astype(ml_dtypes.bfloat16)
        ST8W = np.ascontiguousarray(
            oh.transpose(0, 3, 1, 2).reshape(NCORES, P, NCH * P)
        ).astype(ml_dtypes.bfloat16)
        # wrapped int16 indices per gather op, [128, NOPS*64]
        IDXW = np.zeros((NCORES, P, NOPS * GRP * 8), dtype=np.int16)
        for c in range(NCORES):
            for o, (c0, ncg, t) in enumerate(ops):
                iv = SRCK[c, c0 * P:(c0 + ncg) * P] - (HALF if t else 0)
                w = iv.reshape(-1, 16).T.astype(np.int16)  # [16, n/16]
                IDXW[c, :, o * GRP * 8: o * GRP * 8 + w.shape[1]] = \
                    np.tile(w, (8, 1))
        return dict(NCH=NCH, meta=meta, ops=ops, NOPS=NOPS,
                    Ktot=[int(Ka[s] + Kb[s]) for s in range(NBLK)],
                    S8W=S8W, ST8W=ST8W, IDXW=IDXW)

    L1 = build_layer(1)
    L2 = build_layer(2)

    # per-core own-node x slice (transposed, padded): alpha_dst + self loops
    xTself = np.zeros((NCORES, D, NROWC), dtype=np.float32)
    for c in range(NCORES):
        hi = min(c * NPC + NROWC, N)
        xTself[c, :, :hi - c * NPC] = x[c * NPC:hi].T

    xT = np.zeros((D, NROW1), dtype=np.float32)
    xT[:, :N] = x.T
    # rhs1 = [W1 | per-head W1@as1 (3) | per-head W1@ad1 (3)]  [128, 198]
    as_cols = np.stack([W1[:, h * HID:(h + 1) * HID] @ as1[h]
                        for h in range(H)], axis=1)
    ad_cols = np.stack([W1[:, h * HID:(h + 1) * HID] @ ad1[h]
                        for h in range(H)], axis=1)
    RHS1 = np.ascontiguousarray(
        np.concatenate([W1, as_cols, ad_cols], axis=1).astype(np.float32))
    # rhs2 = [W2 | W2@as2 | W2@ad2]  [192, 66]
    RHS2 = np.ascontiguousarray(np.concatenate(
        [W2, (W2 @ as2[0])[:, None], (W2 @ ad2[0])[:, None]],
        axis=1).astype(np.float32))

    shared = {
        "xT": xT,
        "RHS1": RHS1,
        "RHS2lo": np.ascontiguousarray(RHS2[:P]),
        "RHS2hi": np.ascontiguousarray(RHS2[P:]),
        "B1": np.ascontiguousarray(np.broadcast_to(b1, (P, F1))),
        "B2": np.ascontiguousarray(np.broadcast_to(b2, (P, F2))),
    }
    percore = []
    for c in range(NCORES):
        percore.append({
            "S8W1": L1["S8W"][c], "ST8W1": L1["ST8W"][c],
            "IDXW1": L1["IDXW"][c],
            "S8W2": L2["S8W"][c], "ST8W2": L2["ST8W"][c],
            "IDXW2": L2["IDXW"][c],
            "xTself": xTself[c],
        })
    key = (tuple(L1["Ktot"]), tuple(x[0] for x in L1["ops"]),
           tuple(x[1] for x in L1["ops"]), tuple(x[2] for x in L1["ops"]),
           tuple(L2["Ktot"]), tuple(x[0] for x in L2["ops"]),
           tuple(x[1] for x in L2["ops"]), tuple(x[2] for x in L2["ops"]))
    return key, (L1, L2), shared, percore


def _ap_view(ap, extra_offset, free_dims):
    import concourse.bass as bass

    return bass.AP(
        tensor=ap.tensor, offset=ap.offset + extra_offset,
        ap=[list(ap.ap[0])] + [list(d) for d in free_dims],
    )


def _dram_rows_3d(t, row0, nrows_inner, nblocks, width):
    """AP over DRAM tensor t rows [row0, row0+nblocks*nrows_inner) viewed as
    [nrows_inner (partition), nblocks, width]."""
    base = t[row0:row0 + nrows_inner, :]
    return _ap_view(base, 0,
                    [[nrows_inner * width, nblocks], [1, width]])


def _build(L1, L2):
    import concourse.bass as bass
    import concourse.bacc as bacc
    import concourse.tile as tile
    from concourse import mybir
    from concourse.masks import make_identity
    from concourse.library_config import mlp
    from contextlib import ExitStack

    f32 = mybir.dt.float32
    bf16 = mybir.dt.bfloat16
    i16 = mybir.dt.int16
    AT = mybir.ActivationFunctionType
    OP = mybir.AluOpType

    nc = bacc.Bacc("TRN2", target_bir_lowering=False, debug=False,
                   num_devices=NCORES, num_swdge_queues=4,
                   dynamic_dma_scratch_size=49152)

    xT = nc.dram_tensor("xT", [D, NROW1], f32, kind="ExternalInput")
    RHS1 = nc.dram_tensor("RHS1", [D, F1 + 6], f32, kind="ExternalInput")
    RHS2lo = nc.dram_tensor("RHS2lo", [P, F2 + 2], f32, kind="ExternalInput")
    RHS2hi = nc.dram_tensor("RHS2hi", [F1 - P, F2 + 2], f32,
                            kind="ExternalInput")
    B1 = nc.dram_tensor("B1", [P, F1], f32, kind="ExternalInput")
    B2 = nc.dram_tensor("B2", [P, F2], f32, kind="ExternalInput")
    S8W1 = nc.dram_tensor("S8W1", [P, L1["NCH"] * P], bf16,
                          kind="ExternalInput")
    ST8W1 = nc.dram_tensor("ST8W1", [P, L1["NCH"] * P], bf16,
                           kind="ExternalInput")
    IDXW1 = nc.dram_tensor("IDXW1", [P, L1["NOPS"] * GRP * 8], i16,
                           kind="ExternalInput")
    S8W2 = nc.dram_tensor("S8W2", [P, L2["NCH"] * P], bf16,
                          kind="ExternalInput")
    ST8W2 = nc.dram_tensor("ST8W2", [P, L2["NCH"] * P], bf16,
                           kind="ExternalInput")
    IDXW2 = nc.dram_tensor("IDXW2", [P, L2["NOPS"] * GRP * 8], i16,
                           kind="ExternalInput")
    XTS = nc.dram_tensor("xTself", [D, NROWC], f32, kind="ExternalInput")
    OUT = nc.dram_tensor("out", [NROWC, F2], f32, kind="ExternalOutput")

    G1a = nc.dram_tensor("G1a", [HALF, G1W], bf16, kind="Internal")
    G1b = nc.dram_tensor("G1b", [NROW1 - HALF, G1W], bf16, kind="Internal")
    G2L = nc.dram_tensor("G2L", [NROWC, G2W], bf16, kind="Internal")
    G2F = nc.dram_tensor("G2F", [NROWC * NCORES, G2W], bf16,
                         addr_space="Shared", kind="Internal")

    with tile.TileContext(nc) as tc, ExitStack() as ctx:
        consts = ctx.enter_context(tc.tile_pool(name="consts", bufs=1))
        sbA = ctx.enter_context(tc.tile_pool(name="sbA", bufs=3))
        psA = ctx.enter_context(tc.tile_pool(name="psA", bufs=3, space="PSUM"))
        psum = ctx.enter_context(tc.tile_pool(name="psum", bufs=2, space="PSUM"))
        psad = ctx.enter_context(tc.tile_pool(name="psad", bufs=1, space="PSUM"))
        pst = ctx.enter_context(tc.tile_pool(name="pst", bufs=2, space="PSUM"))
        gpool = ctx.enter_context(tc.tile_pool(name="gpool", bufs=6))
        fpool = ctx.enter_context(tc.tile_pool(name="fpool", bufs=6))
        spool = ctx.enter_context(tc.tile_pool(name="spool", bufs=6))
        ipool = ctx.enter_context(tc.tile_pool(name="ipool", bufs=6))
        epool = ctx.enter_context(tc.tile_pool(name="epool", bufs=4))

        nc.gpsimd.load_library(mlp)

        # ---------------- constants / weight prep ----------------
        b1t = consts.tile([P, F1], f32)
        nc.sync.dma_start(out=b1t[:], in_=B1[:])
        b2t = consts.tile([P, F2], f32)
        nc.sync.dma_start(out=b2t[:], in_=B2[:])
        ident = consts.tile([P, P], f32)
        make_identity(nc, ident[:])
        rhs1 = consts.tile([P, F1 + 6], f32)
        nc.sync.dma_start(out=rhs1[:], in_=RHS1[:])
        rhs2_lo = consts.tile([P, F2 + 2], f32)
        nc.sync.dma_start(out=rhs2_lo[:], in_=RHS2lo[:])
        rhs2_hi = consts.tile([F1 - P, F2 + 2], f32)
        nc.sync.dma_start(out=rhs2_hi[:], in_=RHS2hi[:])
        xtself = consts.tile([P, NROWC], f32)
        nc.sync.dma_start(out=xtself[:], in_=XTS[:])
        # layer-2 per-slot stashes (filled by epi1)
        g2self = consts.tile([P, NBLK, G2W], bf16)
        adb2_all = consts.tile([P, NBLK], f32)

        # ---------------- stage A (G1b tiles first) ----------------
        tiles = list(range(HALF // P, NT)) + list(range(HALF // P))
        groups = []
        i = 0
        while i < len(tiles):
            j = i
            while (j < len(tiles) and j - i < TBATCH
                   and tiles[j] == tiles[i] + (j - i)):
                j += 1
            groups.append((tiles[i], j - i))
            i = j
        for t0, nb in groups:
            xt4 = sbA.tile([P, TBATCH * P], f32, tag="xt")
            nc.sync.dma_start(out=xt4[:, :nb * P],
                              in_=xT[:, t0 * P:(t0 + nb) * P])
            gbf4 = sbA.tile([P, TBATCH, G1W], bf16, tag="gbf")
            for j in range(nb):
                pa = psA.tile([P, 200], f32, tag="mmA")
                nc.tensor.matmul(out=pa[:, :F1 + 3],
                                 lhsT=xt4[:, j * P:(j + 1) * P],
                                 rhs=rhs1[:, :F1 + 3], start=True, stop=True)
                nc.scalar.activation(out=gbf4[:, j, :F1], in_=pa[:, :F1],
                                     func=AT.Copy)
                gf32 = gbf4[:].bitcast(f32)
                nc.vector.tensor_copy(out=gf32[:, j, 96:99],
                                      in_=pa[:, F1:F1 + 3])
            if t0 >= HALF // P:
                dst_ap = _dram_rows_3d(G1b, (t0 - HALF // P) * P, P, nb, G1W)
            else:
                dst_ap = _dram_rows_3d(G1a, t0 * P, P, nb, G1W)
            nc.scalar.dma_start(out=dst_ap, in_=gbf4[:, :nb, :])

        # ---------------- generic edge phase ----------------
        def edge_layer(LM, TBLa, TBLb, width, nfeat, as_f32col, s8w, st8w,
                       idxwi, ps_width, nheads, new_slot, slot_epilogue):
            NCH = LM["NCH"]
            meta = LM["meta"]
            ops = LM["ops"]
            Ktot = LM["Ktot"]
            ps_cur = [None]
            adb_cur = [None]
            fw = nfeat + nheads  # F8 row width

            for o, (c0, ncg, tb) in enumerate(ops):
                idxt = ipool.tile([P, GRP * 8], i16, tag="idxt", name="idxt")
                nc.sync.dma_start(
                    out=idxt[:, :ncg * 8],
                    in_=idxwi[:, o * GRP * 8:o * GRP * 8 + ncg * 8])
                s8t = spool.tile([P, GRP * P], bf16, tag="s8", name="s8t")
                nc.sync.dma_start(
                    out=s8t[:, :ncg * P],
                    in_=s8w[:, c0 * P:(c0 + ncg) * P])
                st8 = spool.tile([P, GRP * P], bf16, tag="st8", name="st8")
                nc.scalar.dma_start(
                    out=st8[:, :ncg * P],
                    in_=st8w[:, c0 * P:(c0 + ncg) * P])
                grow = gpool.tile([P, GRP, width], bf16, tag="grow",
                                  name="grow")
                nidx = ncg * P
                nc.gpsimd.dma_gather(
                    grow[:, :ncg, :], (TBLb if tb else TBLa)[:],
                    idxt[:, :ncg * 8], nidx, nidx, width,
                    queue_num=o % 4)
                # alpha_dst expansion: adp[e, h] = ST8^T x adb
                adp = psad.tile([P, 36], f32, tag="adp", name="adp")
                for jj in range(ncg):
                    s, k, _tb2 = meta[c0 + jj]
                    if k == 0:
                        new_slot(s, ps_cur, adb_cur, adp)
                    nc.tensor.matmul(
                        out=adp[:, jj * nheads:(jj + 1) * nheads],
                        lhsT=st8[:, jj * P:(jj + 1) * P],
                        rhs=adb_cur[0][:, :nheads],
                        start=True, stop=True)
                # logits -> exp -> weighted features
                growf = grow[:].bitcast(f32)
                t8 = epool.tile([P, GRP * nheads], f32, tag="t8", name="t8")
                nc.vector.tensor_tensor(
                    out=_ap_view(t8[:], 0, [[nheads, ncg], [1, nheads]]),
                    in0=_ap_view(growf, as_f32col,
                                 [[width // 2, ncg], [1, nheads]]),
                    in1=_ap_view(adp[:], 0, [[nheads, ncg], [1, nheads]]),
                    op=OP.add)
                # exp(lrelu(t)) == max(exp(t), exp(SLOPE*t)) exactly
                e2 = epool.tile([P, GRP * nheads], f32, tag="r8", name="e2")
                nc.scalar.activation(out=e2[:, :ncg * nheads],
                                     in_=t8[:, :ncg * nheads],
                                     func=AT.Exp, scale=SLOPE)
                F8 = fpool.tile([P, GRP * fw], bf16, tag="f8", name="f8")
                nc.scalar.activation(
                    out=_ap_view(F8[:], nfeat, [[fw, ncg], [1, nheads]]),
                    in_=_ap_view(t8[:], 0, [[nheads, ncg], [1, nheads]]),
                    func=AT.Exp)
                nc.vector.tensor_tensor(
                    out=_ap_view(F8[:], nfeat, [[fw, ncg], [1, nheads]]),
                    in0=_ap_view(F8[:], nfeat, [[fw, ncg], [1, nheads]]),
                    in1=_ap_view(e2[:], 0, [[nheads, ncg], [1, nheads]]),
                    op=OP.max)
                hd = nfeat // nheads
                nc.vector.tensor_tensor(
                    out=_ap_view(F8[:], 0, [[fw, ncg], [hd, nheads], [1, hd]]),
                    in0=_ap_view(grow[:], 0,
                                 [[width, ncg], [hd, nheads], [1, hd]]),
                    in1=_ap_view(F8[:], nfeat,
                                 [[fw, ncg], [1, nheads], [0, hd]]),
                    op=OP.mult)
                for jj in range(ncg):
                    s, k, _tb2 = meta[c0 + jj]
                    nc.tensor.matmul(
                        out=ps_cur[0][:, :ps_width],
                        lhsT=s8t[:, jj * P:(jj + 1) * P],
                        rhs=F8[:, jj * fw:jj * fw + ps_width],
                        start=(k == 0), stop=(k == Ktot[s] - 1))
                    if k == Ktot[s] - 1:
                        slot_epilogue(s, ps_cur[0])

        # ---------------- layer 1 slot hooks ----------------
        def new_slot1(s, ps_cur, adb_cur, adp):
            nc.tensor.matmul(out=adp[:, 32:32 + H],
                             lhsT=xtself[:, s * P:(s + 1) * P],
                             rhs=rhs1[:, F1 + 3:F1 + 6], start=True, stop=True)
            adbh = epool.tile([P, 4], bf16, tag="adbh", name="adbh")
            nc.vector.tensor_copy(out=adbh[:, :H], in_=adp[:, 32:32 + H])
            adb_cur[0] = adbh
            ps_cur[0] = psum.tile([P, 200], f32, tag="mm", name="ps_slot")

        # L1 epilogue: self contribution + h -> transpose -> G2 rows + stash
        def epi1(s, ps):
            # self loop: xw_self (+as/ad) for the slot's own 128 nodes
            pw = psA.tile([P, 200], f32, tag="mmA", name="pw_self")
            nc.tensor.matmul(out=pw[:, :F1 + 6],
                             lhsT=xtself[:, s * P:(s + 1) * P],
                             rhs=rhs1[:], start=True, stop=True)
            aw = epool.tile([P, 6], f32, tag="aw", name="aw")
            nc.vector.tensor_copy(out=aw[:], in_=pw[:, F1:F1 + 6])
            ts = epool.tile([P, H], f32, tag="ts", name="ts")
            nc.vector.tensor_tensor(out=ts[:], in0=aw[:, 0:3],
                                    in1=aw[:, 3:6], op=OP.add)
            es1 = epool.tile([P, H], f32, tag="es1", name="es1")
            nc.scalar.activation(out=es1[:], in_=ts[:], func=AT.Exp,
                                 scale=SLOPE)
            es2 = epool.tile([P, H], f32, tag="es2", name="es2")
            nc.scalar.activation(out=es2[:], in_=ts[:], func=AT.Exp)
            esm = epool.tile([P, H], f32, tag="esm", name="esm")
            nc.vector.tensor_tensor(out=esm[:], in0=es1[:], in1=es2[:],
                                    op=OP.max)
            fs = epool.tile([P, F1 + H], f32, tag="fs", name="fs")
            nc.vector.tensor_tensor(
                out=_ap_view(fs[:], 0, [[HID, H], [1, HID]]),
                in0=_ap_view(pw[:, :F1], 0, [[HID, H], [1, HID]]),
                in1=_ap_view(esm[:], 0, [[1, H], [0, HID]]),
                op=OP.mult)
            nc.vector.tensor_copy(out=fs[:, F1:F1 + H], in_=esm[:])
            tot = epool.tile([P, F1 + H], f32, tag="tot", name="tot")
            nc.vector.tensor_tensor(out=tot[:], in0=ps[:, :F1 + H],
                                    in1=fs[:], op=OP.add)
            # normalize + bias + relu
            rc = epool.tile([P, H], f32, tag="rc", name="rc")
            nc.vector.tensor_scalar_add(out=rc[:], in0=tot[:, F1:F1 + H],
                                        scalar1=EPS)
            rc2 = epool.tile([P, H], f32, tag="rc2", name="rc2")
            nc.vector.reciprocal(out=rc2[:], in_=rc[:])
            hm = epool.tile([P, F1], f32, tag="hm", name="hm")
            nc.vector.tensor_tensor(
                out=_ap_view(hm[:], 0, [[HID, H], [1, HID]]),
                in0=_ap_view(tot[:, :F1], 0, [[HID, H], [1, HID]]),
                in1=_ap_view(rc2[:], 0, [[1, H], [0, HID]]),
                op=OP.mult)
            hb = epool.tile([P, F1], f32, tag="hb", name="hb")
            nc.vector.tensor_tensor(out=hb[:], in0=hm[:], in1=b1t[:], op=OP.add)
            hr = epool.tile([P, F1], f32, tag="hr", name="hr")
            nc.scalar.activation(out=hr[:], in_=hb[:], func=AT.Relu)
            pt1 = pst.tile([P, P], f32, tag="tr", name="pt1")
            nc.tensor.transpose(out=pt1[:], in_=hr[:, :P], identity=ident[:])
            pt2 = pst.tile([P, P], f32, tag="tr", name="pt2")
            nc.tensor.transpose(out=pt2[0:F1 - P, :], in_=hr[:, P:F1],
                                identity=ident[:])
            ht1 = epool.tile([P, P], f32, tag="ht1", name="ht1")
            nc.vector.tensor_copy(out=ht1[:], in_=pt1[:])
            ht2 = epool.tile([F1 - P, P], f32, tag="ht2", name="ht2")
            nc.vector.tensor_copy(out=ht2[:], in_=pt2[0:F1 - P, :])
            pg = psA.tile([P, 200], f32, tag="mmA", name="pg")
            nc.tensor.matmul(out=pg[:, :F2 + 2], lhsT=ht1[:], rhs=rhs2_lo[:],
                             start=True, stop=False)
            nc.tensor.matmul(out=pg[:, :F2 + 2], lhsT=ht2[:], rhs=rhs2_hi[:],
                             start=False, stop=True)
            g2 = epool.tile([P, G2W], bf16, tag="g2", name="g2")
            nc.vector.tensor_copy(out=g2[:, :F2], in_=pg[:, :F2])
            g2f = g2[:].bitcast(f32)
            nc.vector.tensor_copy(out=g2f[:, 32:33], in_=pg[:, F2:F2 + 1])
            nc.vector.tensor_copy(out=g2self[:, s, :], in_=g2[:])
            nc.vector.tensor_copy(out=adb2_all[:, s:s + 1],
                                  in_=pg[:, F2 + 1:F2 + 2])
            nc.sync.dma_start(out=G2L[s * P:(s + 1) * P, :], in_=g2[:])

        edge_layer(L1, G1a, G1b, G1W, F1, 96, S8W1, ST8W1, IDXW1,
                   F1 + H, H, new_slot1, epi1)

        # ---------------- AllGather ----------------
        nc.gpsimd.collective_compute(
            "AllGather", mybir.AluOpType.bypass,
            replica_groups=[list(range(NCORES))],
            ins=[G2L.ap().opt()], outs=[G2F.ap().opt()])

        # ---------------- layer 2 ----------------
        def new_slot2(s, ps_cur, adb_cur, adp):
            adbh = epool.tile([P, 4], bf16, tag="adbh", name="adbh2")
            nc.vector.tensor_copy(out=adbh[:, :1], in_=adb2_all[:, s:s + 1])
            adb_cur[0] = adbh
            ps_cur[0] = psum.tile([P, 200], f32, tag="mm", name="ps_slot2")

        def epi2(s, ps):
            g2sf = g2self[:].bitcast(f32)
            ts = epool.tile([P, 1], f32, tag="tsB", name="tsB")
            nc.vector.tensor_tensor(
                out=ts[:], in0=g2sf[:, s, 32:33],
                in1=adb2_all[:, s:s + 1], op=OP.add)
            es1 = epool.tile([P, 1], f32, tag="es1B", name="es1B")
            nc.scalar.activation(out=es1[:], in_=ts[:], func=AT.Exp,
                                 scale=SLOPE)
            es2 = epool.tile([P, 1], f32, tag="es2B", name="es2B")
            nc.scalar.activation(out=es2[:], in_=ts[:], func=AT.Exp)
            esm = epool.tile([P, 1], f32, tag="esmB", name="esmB")
            nc.vector.tensor_tensor(out=esm[:], in0=es1[:], in1=es2[:],
                                    op=OP.max)
            fs = epool.tile([P, F2 + 1], f32, tag="fsB", name="fsB")
            nc.vector.tensor_tensor(out=fs[:, :F2],
                                    in0=g2self[:, s, :F2],
                                    in1=esm[:].to_broadcast([P, F2]),
                                    op=OP.mult)
            nc.vector.tensor_copy(out=fs[:, F2:F2 + 1], in_=esm[:])
            tot = epool.tile([P, F2 + 1], f32, tag="totB", name="totB")
            nc.vector.tensor_tensor(out=tot[:], in0=ps[:, :F2 + 1],
                                    in1=fs[:], op=OP.add)
            rc = epool.tile([P, 1], f32, tag="rcB", name="rcB")
            nc.vector.tensor_scalar_add(out=rc[:], in0=tot[:, F2:F2 + 1],
                                        scalar1=EPS)
            rc2 = epool.tile([P, 1], f32, tag="rcB2", name="rcB2")
            nc.vector.reciprocal(out=rc2[:], in_=rc[:])
            om = epool.tile([P, F2], f32, tag="om", name="om")
            nc.vector.tensor_tensor(out=om[:], in0=tot[:, :F2],
                                    in1=rc2[:].to_broadcast([P, F2]),
                                    op=OP.mult)
            ob = epool.tile([P, F2], f32, tag="ob", name="ob")
            nc.vector.tensor_tensor(out=ob[:], in0=om[:], in1=b2t[:], op=OP.add)
            orl = epool.tile([P, F2], f32, tag="orl", name="orl")
            nc.scalar.activation(out=orl[:], in_=ob[:], func=AT.Relu)
            nc.sync.dma_start(out=OUT[s * P:(s + 1) * P, :], in_=orl[:])

        # G2F views for the two index halves (offsets stay < 2^24 bytes)
        g2fa = G2F[0:HALF, :]
        g2fb = G2F[HALF:NROWC * NCORES, :]
        edge_layer(L2, g2fa, g2fb, G2W, F2, 32, S8W2, ST8W2, IDXW2,
                   F2 + 1, 1, new_slot2, epi2)

    nc.compile()
    return nc


def _get_compiled(key, layers):
    if key not in _compiled:
        _compiled[key] = _build(layers[0], layers[1])
    return _compiled[key]


def run(inputs, **runkw):
    from concourse import bass_utils

    key, layers, shared, percore = _host_prep(inputs)
    nc = _get_compiled(key, layers)
    in_maps = []
    for c in range(NCORES):
        m = dict(shared)
        m.update(percore[c])
        in_maps.append(m)
    res = bass_utils.run_bass_kernel_spmd(
        nc, in_maps, core_ids=list(range(NCORES)), **runkw)
    return res


def assemble(results):
    out = np.empty((N, F2), dtype=np.float32)
    for c in range(NCORES):
        out[c * NPC:(c + 1) * NPC] = results[c]["out"][:NPC]
    return out


def kernel(**inputs):
    res = run(inputs)
    return assemble(res.results)


# revision 12
# speedup vs baseline: 1.0221x; 1.0221x over previous
"""GAT 2-layer kernel for Trainium2, 8 NeuronCores (SPMD, dst-sharded).

Strategy (v4):
  - Destination-node sharding: core c owns nodes [c*6250,(c+1)*6250); non-self
    edges bucketed into per-128-dst-node "slots", padded to 128-edge chunks.
  - Stage A (replicated): per 4x128-node group one 256KB read of xT, four
    matmuls [x@W1 (192) | alpha_src (3)] -> bf16 gather table G1 (512B rows:
    192 bf16 xw + 3 f32 alpha_src bit-packed + pad), one 256KB batched write.
    G1 split into two <=32768-row tensors (dma_gather int16 index limit).
  - One-hot matrices S8 (e->d) and its transpose ST8 are STATIC functions of
    the edge structure: precomputed host-side as bf16, streamed from HBM per
    op group (replaces on-device DVE compares + PE broadcast matmuls).
  - alpha_dst per slot computed locally from a per-core xTself input slice
    (PE matmul vs rhs1 ad-columns) -- no AD tables, no indirect gathers.
  - Self-loop edges excluded from the gather stream; per-slot self
    contribution (exp(lrelu(as+ad)) * xw_self) computed from xTself / the
    layer-1 epilogue stash and DVE-added into the slot PSUM at epilogue.
  - Edge phase per layer: per <=8-chunk group one dma_gather (1024 rows/op)
    pulls source rows; adp = ST8 x adb expands alpha_dst edge-wise; per-chunk
    segment matmul S8^T x F8 accumulates exp-weighted features + softmax
    denominators in per-slot PSUM (normalization pulled out of the sum).
  - Per-slot epilogue: h = relu((sum+self)/(denom+eps) + bias1); PE-transpose
    h, emit G2 rows [h@W2 (64) bf16 | as2 f32], stash ad2/g2 rows in SBUF for
    layer-2 self/alpha_dst; single AllGather of G2; layer 2 repeats the edge
    phase (1 head) against G2F views.
"""
import sys

sys.path.insert(0, "/opt/trn_rl_repo")
import numpy as np
import ml_dtypes

N = 50000
D = 128
HID = 64
H = 3
F1 = 192
F2 = 64
NCORES = 8
NPC = N // NCORES          # 6250 nodes per core
P = 128
NBLK = (NPC + P - 1) // P  # 49 slots per core
NT = (N + P - 1) // P      # 391 stage-A node tiles
NROW1 = NT * P             # 50048 G1 rows
HALF = 32768               # dma_gather int16 index limit
G1W = 256                  # bf16 cols: xw(192) | as f32 x3 (bf16 192:198) | pad
G2W = 128                  # bf16 cols: xw2(64) | as2 f32 (bf16 64:66) | pad
NROWC = NBLK * P           # 6272 rows per core shard
SLOPE = 0.2
EPS = 1e-16
GRP = 8                    # max chunks per dma_gather / op group
TBATCH = 4                 # stage-A tiles per batched iteration

_compiled = {}


def _chunkize(src_key, dst):
    """Per (core, slot, table-half) chunk counts, maxed over cores."""
    core = dst // NPC
    slot = (dst % NPC) // P
    half = (src_key >= HALF).astype(np.int64)
    counts = np.zeros((NCORES, NBLK, 2), dtype=np.int64)
    np.add.at(counts, (core, slot, half), 1)
    Ka = np.ceil(counts[:, :, 0] / P).astype(np.int64).max(axis=0)
    Kb = np.ceil(counts[:, :, 1] / P).astype(np.int64).max(axis=0)
    return Ka, Kb


def _host_prep(inputs):
    x = np.asarray(inputs["x"], dtype=np.float32)
    ei = np.asarray(inputs["edge_index"])
    W1 = np.asarray(inputs["W1"], dtype=np.float32)
    as1 = np.asarray(inputs["att_src1"], dtype=np.float32)
    ad1 = np.asarray(inputs["att_dst1"], dtype=np.float32)
    b1 = np.asarray(inputs["bias1"], dtype=np.float32)
    W2 = np.asarray(inputs["W2"], dtype=np.float32)
    as2 = np.asarray(inputs["att_src2"], dtype=np.float32)
    ad2 = np.asarray(inputs["att_dst2"], dtype=np.float32)
    b2 = np.asarray(inputs["bias2"], dtype=np.float32)

    # self-loops are handled per-slot on device; only real edges here
    src = ei[0].astype(np.int64)
    dst = ei[1].astype(np.int64)
    order = np.argsort(dst, kind="stable")
    src = src[order]
    dst = dst[order]
    g2row = (src // NPC) * NROWC + (src % NPC)

    Ka1, Kb1 = _chunkize(src, dst)
    Ka2, Kb2 = _chunkize(g2row, dst)

    def build_layer(key):
        Ka, Kb = (Ka1, Kb1) if key == 1 else (Ka2, Kb2)
        skey = src if key == 1 else g2row
        NCH = int((Ka + Kb).sum())
        assert (Ka + Kb).min() >= 1, "empty slot: epilogue would be skipped"
        # chunk meta: (slot, k_in_slot, table) in processing order
        # b-table chunks first: their gathers only depend on the (smaller,
        # first-written) G1b table, overlapping the tail of stage A
        meta = []
        for s in range(NBLK):
            k = 0
            for _ in range(int(Kb[s])):
                meta.append((s, k, 1)); k += 1
            for _ in range(int(Ka[s])):
                meta.append((s, k, 0)); k += 1
        # gather ops: runs of <=GRP same-table consecutive chunks
        ops = []   # (chunk_start, n_chunks, table)
        i = 0
        while i < NCH:
            t = meta[i][2]
            j = i
            while j < NCH and j - i < GRP and meta[j][2] == t:
                j += 1
            ops.append((i, j - i, t))
            i = j
        NOPS = len(ops)

        EPAD = NCH * P
        SRCK = np.zeros((NCORES, EPAD), dtype=np.int64)
        DREL = np.full((NCORES, EPAD), 255.0, dtype=np.float32)
        for c in range(NCORES):
            base_node = c * NPC
            cb = 0
            for s in range(NBLK):
                blo = base_node + s * P
                bhi = min(blo + P, base_node + NPC)
                lo = np.searchsorted(dst, blo, side="left")
                hi = np.searchsorted(dst, bhi, side="left")
                sk = skey[lo:hi]
                dr = (dst[lo:hi] - blo).astype(np.float32)
                a_mask = sk < HALF
                for which, KK, pad in ((~a_mask, Kb[s], HALF),
                                       (a_mask, Ka[s], 0)):
                    cnt = int(which.sum())
                    pos = cb * P
                    SRCK[c, pos:pos + cnt] = sk[which]
                    # pad indices must stay valid for the table half
                    SRCK[c, pos + cnt:(cb + int(KK)) * P] = pad
                    DREL[c, pos:pos + cnt] = dr[which]
                    cb += int(KK)
        # static one-hot matrices, bf16:
        #   S8W[c, e, ch*128+d] = (DREL[ch, e] == d)   (segment-matmul lhsT)
        #   ST8W[c, d, ch*128+e] = same, transposed    (alpha-dst-expand lhsT)
        drel_ch = DREL.reshape(NCORES, NCH, P)
        oh = (drel_ch[:, :, :, None] ==
              np.arange(P, dtype=np.float32)[None, None, None, :])
        S8W = np.ascontiguousarray(
            oh.transpose(0, 2, 1, 3).reshape(NCORES, P, NCH * P)
        ).astype(ml_dtypes.bfloat16)
        ST8W = np.ascontiguousarray(
            oh.transpose(0, 3, 1, 2).reshape(NCORES, P, NCH * P)
        ).astype(ml_dtypes.bfloat16)
        # wrapped int16 indices per gather op, [128, NOPS*64]
        IDXW = np.zeros((NCORES, P, NOPS * GRP * 8), dtype=np.int16)
        for c in range(NCORES):
            for o, (c0, ncg, t) in enumerate(ops):
                iv = SRCK[c, c0 * P:(c0 + ncg) * P] - (HALF if t else 0)
                w = iv.reshape(-1, 16).T.astype(np.int16)  # [16, n/16]
                IDXW[c, :, o * GRP * 8: o * GRP * 8 + w.shape[1]] = \
                    np.tile(w, (8, 1))
        return dict(NCH=NCH, meta=meta, ops=ops, NOPS=NOPS,
                    Ktot=[int(Ka[s] + Kb[s]) for s in range(NBLK)],
                    S8W=S8W, ST8W=ST8W, IDXW=IDXW)

    L1 = build_layer(1)
    L2 = build_layer(2)

    # per-core own-node x slice (transposed, padded): alpha_dst + self loops
    xTself = np.zeros((NCORES, D, NROWC), dtype=ml_dtypes.bfloat16)
    for c in range(NCORES):
        hi = min(c * NPC + NROWC, N)
        xTself[c, :, :hi - c * NPC] = x[c * NPC:hi].T.astype(ml_dtypes.bfloat16)

    xT = np.zeros((D, NROW1), dtype=ml_dtypes.bfloat16)
    xT[:, :N] = x.T.astype(ml_dtypes.bfloat16)
    # rhs1 = [W1 | per-head W1@as1 (3) | per-head W1@ad1 (3)]  [128, 198]
    as_cols = np.stack([W1[:, h * HID:(h + 1) * HID] @ as1[h]
                        for h in range(H)], axis=1)
    ad_cols = np.stack([W1[:, h * HID:(h + 1) * HID] @ ad1[h]
                        for h in range(H)], axis=1)
    RHS1 = np.ascontiguousarray(
        np.concatenate([W1, as_cols, ad_cols],
                       axis=1).astype(ml_dtypes.bfloat16))
    # rhs2 = [W2 | W2@as2 | W2@ad2]  [192, 66]
    RHS2 = np.ascontiguousarray(np.concatenate(
        [W2, (W2 @ as2[0])[:, None], (W2 @ ad2[0])[:, None]],
        axis=1).astype(np.float32))

    shared = {
        "xT": xT,
        "RHS1": RHS1,
        "RHS2lo": np.ascontiguousarray(RHS2[:P]),
        "RHS2hi": np.ascontiguousarray(RHS2[P:]),
        "B1": np.ascontiguousarray(np.broadcast_to(b1, (P, F1))),
        "B2": np.ascontiguousarray(np.broadcast_to(b2, (P, F2))),
    }
    percore = []
    for c in range(NCORES):
        percore.append({
            "S8W1": L1["S8W"][c], "ST8W1": L1["ST8W"][c],
            "IDXW1": L1["IDXW"][c],
            "S8W2": L2["S8W"][c], "ST8W2": L2["ST8W"][c],
            "IDXW2": L2["IDXW"][c],
            "xTself": xTself[c],
        })
    key = (tuple(L1["Ktot"]), tuple(x[0] for x in L1["ops"]),
           tuple(x[1] for x in L1["ops"]), tuple(x[2] for x in L1["ops"]),
           tuple(L2["Ktot"]), tuple(x[0] for x in L2["ops"]),
           tuple(x[1] for x in L2["ops"]), tuple(x[2] for x in L2["ops"]))
    return key, (L1, L2), shared, percore


def _ap_view(ap, extra_offset, free_dims):
    import concourse.bass as bass

    return bass.AP(
        tensor=ap.tensor, offset=ap.offset + extra_offset,
        ap=[list(ap.ap[0])] + [list(d) for d in free_dims],
    )


def _dram_rows_3d(t, row0, nrows_inner, nblocks, width):
    """AP over DRAM tensor t rows [row0, row0+nblocks*nrows_inner) viewed as
    [nrows_inner (partition), nblocks, width]."""
    base = t[row0:row0 + nrows_inner, :]
    return _ap_view(base, 0,
                    [[nrows_inner * width, nblocks], [1, width]])


def _build(L1, L2):
    import concourse.bass as bass
    import concourse.bacc as bacc
    import concourse.tile as tile
    from concourse import mybir
    from concourse.masks import make_identity
    from concourse.library_config import mlp
    from contextlib import ExitStack

    f32 = mybir.dt.float32
    bf16 = mybir.dt.bfloat16
    i16 = mybir.dt.int16
    AT = mybir.ActivationFunctionType
    OP = mybir.AluOpType

    nc = bacc.Bacc("TRN2", target_bir_lowering=False, debug=False,
                   num_devices=NCORES, num_swdge_queues=4)

    xT = nc.dram_tensor("xT", [D, NROW1], bf16, kind="ExternalInput")
    RHS1 = nc.dram_tensor("RHS1", [D, F1 + 6], bf16, kind="ExternalInput")
    RHS2lo = nc.dram_tensor("RHS2lo", [P, F2 + 2], f32, kind="ExternalInput")
    RHS2hi = nc.dram_tensor("RHS2hi", [F1 - P, F2 + 2], f32,
                            kind="ExternalInput")
    B1 = nc.dram_tensor("B1", [P, F1], f32, kind="ExternalInput")
    B2 = nc.dram_tensor("B2", [P, F2], f32, kind="ExternalInput")
    S8W1 = nc.dram_tensor("S8W1", [P, L1["NCH"] * P], bf16,
                          kind="ExternalInput")
    ST8W1 = nc.dram_tensor("ST8W1", [P, L1["NCH"] * P], bf16,
                           kind="ExternalInput")
    IDXW1 = nc.dram_tensor("IDXW1", [P, L1["NOPS"] * GRP * 8], i16,
                           kind="ExternalInput")
    S8W2 = nc.dram_tensor("S8W2", [P, L2["NCH"] * P], bf16,
                          kind="ExternalInput")
    ST8W2 = nc.dram_tensor("ST8W2", [P, L2["NCH"] * P], bf16,
                           kind="ExternalInput")
    IDXW2 = nc.dram_tensor("IDXW2", [P, L2["NOPS"] * GRP * 8], i16,
                           kind="ExternalInput")
    XTS = nc.dram_tensor("xTself", [D, NROWC], bf16, kind="ExternalInput")
    OUT = nc.dram_tensor("out", [NROWC, F2], f32, kind="ExternalOutput")

    G1a = nc.dram_tensor("G1a", [HALF, G1W], bf16, kind="Internal")
    G1b = nc.dram_tensor("G1b", [NROW1 - HALF, G1W], bf16, kind="Internal")
    G2L = nc.dram_tensor("G2L", [NROWC, G2W], bf16, kind="Internal")
    G2F = nc.dram_tensor("G2F", [NROWC * NCORES, G2W], bf16,
                         addr_space="Shared", kind="Internal")

    with tile.TileContext(nc) as tc, ExitStack() as ctx:
        consts = ctx.enter_context(tc.tile_pool(name="consts", bufs=1))
        sbA = ctx.enter_context(tc.tile_pool(name="sbA", bufs=3))
        psA = ctx.enter_context(tc.tile_pool(name="psA", bufs=3, space="PSUM"))
        psum = ctx.enter_context(tc.tile_pool(name="psum", bufs=2, space="PSUM"))
        psad = ctx.enter_context(tc.tile_pool(name="psad", bufs=1, space="PSUM"))
        pst = ctx.enter_context(tc.tile_pool(name="pst", bufs=2, space="PSUM"))
        gpool = ctx.enter_context(tc.tile_pool(name="gpool", bufs=6))
        fpool = ctx.enter_context(tc.tile_pool(name="fpool", bufs=6))
        spool = ctx.enter_context(tc.tile_pool(name="spool", bufs=6))
        ipool = ctx.enter_context(tc.tile_pool(name="ipool", bufs=6))
        epool = ctx.enter_context(tc.tile_pool(name="epool", bufs=4))

        nc.gpsimd.load_library(mlp)

        # ---------------- constants / weight prep ----------------
        b1t = consts.tile([P, F1], f32)
        nc.sync.dma_start(out=b1t[:], in_=B1[:])
        b2t = consts.tile([P, F2], f32)
        nc.sync.dma_start(out=b2t[:], in_=B2[:])
        ident = consts.tile([P, P], f32)
        make_identity(nc, ident[:])
        rhs1 = consts.tile([P, F1 + 6], bf16)
        nc.sync.dma_start(out=rhs1[:], in_=RHS1[:])
        rhs2_lo = consts.tile([P, F2 + 2], f32)
        nc.sync.dma_start(out=rhs2_lo[:], in_=RHS2lo[:])
        rhs2_hi = consts.tile([F1 - P, F2 + 2], f32)
        nc.sync.dma_start(out=rhs2_hi[:], in_=RHS2hi[:])
        xtself = consts.tile([P, NROWC], bf16)
        nc.sync.dma_start(out=xtself[:], in_=XTS[:])
        # layer-2 per-slot stashes (filled by epi1)
        g2self = consts.tile([P, NBLK, G2W], bf16)
        adb2_all = consts.tile([P, NBLK], f32)

        # ---------------- stage A (G1b tiles first) ----------------
        tiles = list(range(HALF // P, NT)) + list(range(HALF // P))
        groups = []
        i = 0
        while i < len(tiles):
            j = i
            while (j < len(tiles) and j - i < TBATCH
                   and tiles[j] == tiles[i] + (j - i)):
                j += 1
            groups.append((tiles[i], j - i))
            i = j
        for t0, nb in groups:
            xt4 = sbA.tile([P, TBATCH * P], bf16, tag="xt")
            nc.sync.dma_start(out=xt4[:, :nb * P],
                              in_=xT[:, t0 * P:(t0 + nb) * P])
            gbf4 = sbA.tile([P, TBATCH, G1W], bf16, tag="gbf")
            for j in range(nb):
                pa = psA.tile([P, 200], f32, tag="mmA")
                nc.tensor.matmul(out=pa[:, :F1 + 3],
                                 lhsT=xt4[:, j * P:(j + 1) * P],
                                 rhs=rhs1[:, :F1 + 3], start=True, stop=True)
                nc.scalar.activation(out=gbf4[:, j, :F1], in_=pa[:, :F1],
                                     func=AT.Copy)
                gf32 = gbf4[:].bitcast(f32)
                nc.vector.tensor_copy(out=gf32[:, j, 96:99],
                                      in_=pa[:, F1:F1 + 3])
            if t0 >= HALF // P:
                dst_ap = _dram_rows_3d(G1b, (t0 - HALF // P) * P, P, nb, G1W)
            else:
                dst_ap = _dram_rows_3d(G1a, t0 * P, P, nb, G1W)
            nc.scalar.dma_start(out=dst_ap, in_=gbf4[:, :nb, :])

        # ---------------- generic edge phase ----------------
        def edge_layer(LM, TBLa, TBLb, width, nfeat, as_f32col, s8w, st8w,
                       idxwi, ps_width, nheads, new_slot, slot_epilogue):
            NCH = LM["NCH"]
            meta = LM["meta"]
            ops = LM["ops"]
            Ktot = LM["Ktot"]
            ps_cur = [None]
            adb_cur = [None]
            fw = nfeat + nheads  # F8 row width

            for o, (c0, ncg, tb) in enumerate(ops):
                idxt = ipool.tile([P, GRP * 8], i16, tag="idxt", name="idxt")
                nc.sync.dma_start(
                    out=idxt[:, :ncg * 8],
                    in_=idxwi[:, o * GRP * 8:o * GRP * 8 + ncg * 8])
                s8t = spool.tile([P, GRP * P], bf16, tag="s8", name="s8t")
                nc.sync.dma_start(
                    out=s8t[:, :ncg * P],
                    in_=s8w[:, c0 * P:(c0 + ncg) * P])
                st8 = spool.tile([P, GRP * P], bf16, tag="st8", name="st8")
                nc.scalar.dma_start(
                    out=st8[:, :ncg * P],
                    in_=st8w[:, c0 * P:(c0 + ncg) * P])
                grow = gpool.tile([P, GRP, width], bf16, tag="grow",
                                  name="grow")
                nidx = ncg * P
                nc.gpsimd.dma_gather(
                    grow[:, :ncg, :], (TBLb if tb else TBLa)[:],
                    idxt[:, :ncg * 8], nidx, nidx, width,
                    queue_num=o % 4)
                # alpha_dst expansion: adp[e, h] = ST8^T x adb
                adp = psad.tile([P, 36], f32, tag="adp", name="adp")
                for jj in range(ncg):
                    s, k, _tb2 = meta[c0 + jj]
                    if k == 0:
                        new_slot(s, ps_cur, adb_cur, adp)
                    nc.tensor.matmul(
                        out=adp[:, jj * nheads:(jj + 1) * nheads],
                        lhsT=st8[:, jj * P:(jj + 1) * P],
                        rhs=adb_cur[0][:, :nheads],
                        start=True, stop=True)
                # logits -> exp -> weighted features
                growf = grow[:].bitcast(f32)
                t8 = epool.tile([P, GRP * nheads], f32, tag="t8", name="t8")
                nc.vector.tensor_tensor(
                    out=_ap_view(t8[:], 0, [[nheads, ncg], [1, nheads]]),
                    in0=_ap_view(growf, as_f32col,
                                 [[width // 2, ncg], [1, nheads]]),
                    in1=_ap_view(adp[:], 0, [[nheads, ncg], [1, nheads]]),
                    op=OP.add)
                # exp(lrelu(t)) == max(exp(t), exp(SLOPE*t)) exactly
                e2 = epool.tile([P, GRP * nheads], f32, tag="r8", name="e2")
                nc.scalar.activation(out=e2[:, :ncg * nheads],
                                     in_=t8[:, :ncg * nheads],
                                     func=AT.Exp, scale=SLOPE)
                F8 = fpool.tile([P, GRP * fw], bf16, tag="f8", name="f8")
                nc.scalar.activation(
                    out=_ap_view(F8[:], nfeat, [[fw, ncg], [1, nheads]]),
                    in_=_ap_view(t8[:], 0, [[nheads, ncg], [1, nheads]]),
                    func=AT.Exp)
                nc.vector.tensor_tensor(
                    out=_ap_view(F8[:], nfeat, [[fw, ncg], [1, nheads]]),
                    in0=_ap_view(F8[:], nfeat, [[fw, ncg], [1, nheads]]),
                    in1=_ap_view(e2[:], 0, [[nheads, ncg], [1, nheads]]),
                    op=OP.max)
                hd = nfeat // nheads
                nc.vector.tensor_tensor(
                    out=_ap_view(F8[:], 0, [[fw, ncg], [hd, nheads], [1, hd]]),
                    in0=_ap_view(grow[:], 0,
                                 [[width, ncg], [hd, nheads], [1, hd]]),
                    in1=_ap_view(F8[:], nfeat,
                                 [[fw, ncg], [1, nheads], [0, hd]]),
                    op=OP.mult)
                for jj in range(ncg):
                    s, k, _tb2 = meta[c0 + jj]
                    nc.tensor.matmul(
                        out=ps_cur[0][:, :ps_width],
                        lhsT=s8t[:, jj * P:(jj + 1) * P],
                        rhs=F8[:, jj * fw:jj * fw + ps_width],
                        start=(k == 0), stop=(k == Ktot[s] - 1))
                    if k == Ktot[s] - 1:
                        slot_epilogue(s, ps_cur[0])

        # ---------------- layer 1 slot hooks ----------------
        def new_slot1(s, ps_cur, adb_cur, adp):
            nc.tensor.matmul(out=adp[:, 32:32 + H],
                             lhsT=xtself[:, s * P:(s + 1) * P],
                             rhs=rhs1[:, F1 + 3:F1 + 6], start=True, stop=True)
            adbh = epool.tile([P, 4], bf16, tag="adbh", name="adbh")
            nc.vector.tensor_copy(out=adbh[:, :H], in_=adp[:, 32:32 + H])
            adb_cur[0] = adbh
            ps_cur[0] = psum.tile([P, 200], f32, tag="mm", name="ps_slot")

        # L1 epilogue: self contribution + h -> transpose -> G2 rows + stash
        def epi1(s, ps):
            # self loop: xw_self (+as/ad) for the slot's own 128 nodes
            pw = psA.tile([P, 200], f32, tag="mmA", name="pw_self")
            nc.tensor.matmul(out=pw[:, :F1 + 6],
                             lhsT=xtself[:, s * P:(s + 1) * P],
                             rhs=rhs1[:], start=True, stop=True)
            aw = epool.tile([P, 6], f32, tag="aw", name="aw")
            nc.vector.tensor_copy(out=aw[:], in_=pw[:, F1:F1 + 6])
            ts = epool.tile([P, H], f32, tag="ts", name="ts")
            nc.vector.tensor_tensor(out=ts[:], in0=aw[:, 0:3],
                                    in1=aw[:, 3:6], op=OP.add)
            es1 = epool.tile([P, H], f32, tag="es1", name="es1")
            nc.scalar.activation(out=es1[:], in_=ts[:], func=AT.Exp,
                                 scale=SLOPE)
            es2 = epool.tile([P, H], f32, tag="es2", name="es2")
            nc.scalar.activation(out=es2[:], in_=ts[:], func=AT.Exp)
            esm = epool.tile([P, H], f32, tag="esm", name="esm")
            nc.vector.tensor_tensor(out=esm[:], in0=es1[:], in1=es2[:],
                                    op=OP.max)
            fs = epool.tile([P, F1 + H], f32, tag="fs", name="fs")
            nc.vector.tensor_tensor(
                out=_ap_view(fs[:], 0, [[HID, H], [1, HID]]),
                in0=_ap_view(pw[:, :F1], 0, [[HID, H], [1, HID]]),
                in1=_ap_view(esm[:], 0, [[1, H], [0, HID]]),
                op=OP.mult)
            nc.vector.tensor_copy(out=fs[:, F1:F1 + H], in_=esm[:])
            tot = epool.tile([P, F1 + H], f32, tag="tot", name="tot")
            nc.vector.tensor_tensor(out=tot[:], in0=ps[:, :F1 + H],
                                    in1=fs[:], op=OP.add)
            # normalize + bias + relu
            rc = epool.tile([P, H], f32, tag="rc", name="rc")
            nc.vector.tensor_scalar_add(out=rc[:], in0=tot[:, F1:F1 + H],
                                        scalar1=EPS)
            rc2 = epool.tile([P, H], f32, tag="rc2", name="rc2")
            nc.vector.reciprocal(out=rc2[:], in_=rc[:])
            hm = epool.tile([P, F1], f32, tag="hm", name="hm")
            nc.vector.tensor_tensor(
                out=_ap_view(hm[:], 0, [[HID, H], [1, HID]]),
                in0=_ap_view(tot[:, :F1], 0, [[HID, H], [1, HID]]),
                in1=_ap_view(rc2[:], 0, [[1, H], [0, HID]]),
                op=OP.mult)
            hb = epool.tile([P, F1], f32, tag="hb", name="hb")
            nc.vector.tensor_tensor(out=hb[:], in0=hm[:], in1=b1t[:], op=OP.add)
            hr = epool.tile([P, F1], f32, tag="hr", name="hr")
            nc.scalar.activation(out=hr[:], in_=hb[:], func=AT.Relu)
            pt1 = pst.tile([P, P], f32, tag="tr", name="pt1")
            nc.tensor.transpose(out=pt1[:], in_=hr[:, :P], identity=ident[:])
            pt2 = pst.tile([P, P], f32, tag="tr", name="pt2")
            nc.tensor.transpose(out=pt2[0:F1 - P, :], in_=hr[:, P:F1],
                                identity=ident[:])
            ht1 = epool.tile([P, P], f32, tag="ht1", name="ht1")
            nc.vector.tensor_copy(out=ht1[:], in_=pt1[:])
            ht2 = epool.tile([F1 - P, P], f32, tag="ht2", name="ht2")
            nc.vector.tensor_copy(out=ht2[:], in_=pt2[0:F1 - P, :])
            pg = psA.tile([P, 200], f32, tag="mmA", name="pg")
            nc.tensor.matmul(out=pg[:, :F2 + 2], lhsT=ht1[:], rhs=rhs2_lo[:],
                             start=True, stop=False)
            nc.tensor.matmul(out=pg[:, :F2 + 2], lhsT=ht2[:], rhs=rhs2_hi[:],
                             start=False, stop=True)
            g2 = epool.tile([P, G2W], bf16, tag="g2", name="g2")
            nc.vector.tensor_copy(out=g2[:, :F2], in_=pg[:, :F2])
            g2f = g2[:].bitcast(f32)
            nc.vector.tensor_copy(out=g2f[:, 32:33], in_=pg[:, F2:F2 + 1])
            nc.vector.tensor_copy(out=g2self[:, s, :], in_=g2[:])
            nc.vector.tensor_copy(out=adb2_all[:, s:s + 1],
                                  in_=pg[:, F2 + 1:F2 + 2])
            nc.sync.dma_start(out=G2L[s * P:(s + 1) * P, :], in_=g2[:])

        edge_layer(L1, G1a, G1b, G1W, F1, 96, S8W1, ST8W1, IDXW1,
                   F1 + H, H, new_slot1, epi1)

        # ---------------- AllGather ----------------
        nc.gpsimd.collective_compute(
            "AllGather", mybir.AluOpType.bypass,
            replica_groups=[list(range(NCORES))],
            ins=[G2L.ap().opt()], outs=[G2F.ap().opt()])

        # ---------------- layer 2 ----------------
        def new_slot2(s, ps_cur, adb_cur, adp):
            adbh = epool.tile([P, 4], bf16, tag="adbh", name="adbh2")
            nc.vector.tensor_copy(out=adbh[:, :1], in_=adb2_all[:, s:s + 1])
            adb_cur[0] = adbh
            ps_cur[0] = psum.tile([P, 200], f32, tag="mm", name="ps_slot2")

        def epi2(s, ps):
            g2sf = g2self[:].bitcast(f32)
            ts = epool.tile([P, 1], f32, tag="tsB", name="tsB")
            nc.vector.tensor_tensor(
                out=ts[:], in0=g2sf[:, s, 32:33],
                in1=adb2_all[:, s:s + 1], op=OP.add)
            es1 = epool.tile([P, 1], f32, tag="es1B", name="es1B")
            nc.scalar.activation(out=es1[:], in_=ts[:], func=AT.Exp,
                                 scale=SLOPE)
            es2 = epool.tile([P, 1], f32, tag="es2B", name="es2B")
            nc.scalar.activation(out=es2[:], in_=ts[:], func=AT.Exp)
            esm = epool.tile([P, 1], f32, tag="esmB", name="esmB")
            nc.vector.tensor_tensor(out=esm[:], in0=es1[:], in1=es2[:],
                                    op=OP.max)
            fs = epool.tile([P, F2 + 1], f32, tag="fsB", name="fsB")
            nc.vector.tensor_tensor(out=fs[:, :F2],
                                    in0=g2self[:, s, :F2],
                                    in1=esm[:].to_broadcast([P, F2]),
                                    op=OP.mult)
            nc.vector.tensor_copy(out=fs[:, F2:F2 + 1], in_=esm[:])
            tot = epool.tile([P, F2 + 1], f32, tag="totB", name="totB")
            nc.vector.tensor_tensor(out=tot[:], in0=ps[:, :F2 + 1],
                                    in1=fs[:], op=OP.add)
            rc = epool.tile([P, 1], f32, tag="rcB", name="rcB")
            nc.vector.tensor_scalar_add(out=rc[:], in0=tot[:, F2:F2 + 1],
                                        scalar1=EPS)
            rc2 = epool.tile([P, 1], f32, tag="rcB2", name="rcB2")
            nc.vector.reciprocal(out=rc2[:], in_=rc[:])
            om = epool.tile([P, F2], f32, tag="om", name="om")
            nc.vector.tensor_tensor(out=om[:], in0=tot[:, :F2],
                                    in1=rc2[:].to_broadcast([P, F2]),
                                    op=OP.mult)
            ob = epool.tile([P, F2], f32, tag="ob", name="ob")
            nc.vector.tensor_tensor(out=ob[:], in0=om[:], in1=b2t[:], op=OP.add)
            orl = epool.tile([P, F2], f32, tag="orl", name="orl")
            nc.scalar.activation(out=orl[:], in_=ob[:], func=AT.Relu)
            nc.sync.dma_start(out=OUT[s * P:(s + 1) * P, :], in_=orl[:])

        # G2F views for the two index halves (offsets stay < 2^24 bytes)
        g2fa = G2F[0:HALF, :]
        g2fb = G2F[HALF:NROWC * NCORES, :]
        edge_layer(L2, g2fa, g2fb, G2W, F2, 32, S8W2, ST8W2, IDXW2,
                   F2 + 1, 1, new_slot2, epi2)

    nc.compile()
    return nc


def _get_compiled(key, layers):
    if key not in _compiled:
        _compiled[key] = _build(layers[0], layers[1])
    return _compiled[key]


def run(inputs, **runkw):
    from concourse import bass_utils

    key, layers, shared, percore = _host_prep(inputs)
    nc = _get_compiled(key, layers)
    in_maps = []
    for c in range(NCORES):
        m = dict(shared)
        m.update(percore[c])
        in_maps.append(m)
    res = bass_utils.run_bass_kernel_spmd(
        nc, in_maps, core_ids=list(range(NCORES)), **runkw)
    return res


def assemble(results):
    out = np.empty((N, F2), dtype=np.float32)
    for c in range(NCORES):
        out[c * NPC:(c + 1) * NPC] = results[c]["out"][:NPC]
    return out


def kernel(**inputs):
    res = run(inputs)
    return assemble(res.results)


# revision 14
# speedup vs baseline: 1.0822x; 1.0588x over previous
"""GAT 2-layer kernel for Trainium2, 8 NeuronCores (SPMD, dst-sharded).

Strategy (v4):
  - Destination-node sharding: core c owns nodes [c*6250,(c+1)*6250); non-self
    edges bucketed into per-128-dst-node "slots", padded to 128-edge chunks.
  - Stage A (replicated): per 4x128-node group one 256KB read of xT, four
    matmuls [x@W1 (192) | alpha_src (3)] -> bf16 gather table G1 (512B rows:
    192 bf16 xw + 3 f32 alpha_src bit-packed + pad), one 256KB batched write.
    G1 split into two <=32768-row tensors (dma_gather int16 index limit).
  - One-hot matrices S8 (e->d) and its transpose ST8 are STATIC functions of
    the edge structure: precomputed host-side as bf16, streamed from HBM per
    op group (replaces on-device DVE compares + PE broadcast matmuls).
  - alpha_dst per slot computed locally from a per-core xTself input slice
    (PE matmul vs rhs1 ad-columns) -- no AD tables, no indirect gathers.
  - Self-loop edges excluded from the gather stream; per-slot self
    contribution (exp(lrelu(as+ad)) * xw_self) computed from xTself / the
    layer-1 epilogue stash and DVE-added into the slot PSUM at epilogue.
  - Edge phase per layer: per <=8-chunk group one dma_gather (1024 rows/op)
    pulls source rows; adp = ST8 x adb expands alpha_dst edge-wise; per-chunk
    segment matmul S8^T x F8 accumulates exp-weighted features + softmax
    denominators in per-slot PSUM (normalization pulled out of the sum).
  - Per-slot epilogue: h = relu((sum+self)/(denom+eps) + bias1); PE-transpose
    h, emit G2 rows [h@W2 (64) bf16 | as2 f32], stash ad2/g2 rows in SBUF for
    layer-2 self/alpha_dst; single AllGather of G2; layer 2 repeats the edge
    phase (1 head) against G2F views.
"""
import sys

sys.path.insert(0, "/opt/trn_rl_repo")
import numpy as np
import ml_dtypes

N = 50000
D = 128
HID = 64
H = 3
F1 = 192
F2 = 64
NCORES = 8
NPC = N // NCORES          # 6250 nodes per core
P = 128
NBLK = (NPC + P - 1) // P  # 49 slots per core
NT = (N + P - 1) // P      # 391 stage-A node tiles
NROW1 = NT * P             # 50048 G1 rows
HALF = 32768               # dma_gather int16 index limit
G1W = 256                  # bf16 cols: xw(192) | as f32 x3 (bf16 192:198) | pad
G2W = 128                  # bf16 cols: xw2(64) | as2 f32 (bf16 64:66) | pad
NROWC = NBLK * P           # 6272 rows per core shard
SLOPE = 0.2
EPS = 1e-16
GRP = 8                    # max chunks per dma_gather / op group
TBATCH = 4                 # stage-A tiles per batched iteration

_compiled = {}


def _chunkize(src_key, dst):
    """Per (core, slot, table-half) chunk counts, maxed over cores."""
    core = dst // NPC
    slot = (dst % NPC) // P
    half = (src_key >= HALF).astype(np.int64)
    counts = np.zeros((NCORES, NBLK, 2), dtype=np.int64)
    np.add.at(counts, (core, slot, half), 1)
    Ka = np.ceil(counts[:, :, 0] / P).astype(np.int64).max(axis=0)
    Kb = np.ceil(counts[:, :, 1] / P).astype(np.int64).max(axis=0)
    return Ka, Kb


def _host_prep(inputs):
    x = np.asarray(inputs["x"], dtype=np.float32)
    ei = np.asarray(inputs["edge_index"])
    W1 = np.asarray(inputs["W1"], dtype=np.float32)
    as1 = np.asarray(inputs["att_src1"], dtype=np.float32)
    ad1 = np.asarray(inputs["att_dst1"], dtype=np.float32)
    b1 = np.asarray(inputs["bias1"], dtype=np.float32)
    W2 = np.asarray(inputs["W2"], dtype=np.float32)
    as2 = np.asarray(inputs["att_src2"], dtype=np.float32)
    ad2 = np.asarray(inputs["att_dst2"], dtype=np.float32)
    b2 = np.asarray(inputs["bias2"], dtype=np.float32)

    # self-loops are handled per-slot on device; only real edges here
    src = ei[0].astype(np.int64)
    dst = ei[1].astype(np.int64)
    order = np.argsort(dst, kind="stable")
    src = src[order]
    dst = dst[order]
    g2row = (src // NPC) * NROWC + (src % NPC)

    Ka1, Kb1 = _chunkize(src, dst)
    Ka2, Kb2 = _chunkize(g2row, dst)

    def build_layer(key):
        Ka, Kb = (Ka1, Kb1) if key == 1 else (Ka2, Kb2)
        skey = src if key == 1 else g2row
        NCH = int((Ka + Kb).sum())
        assert (Ka + Kb).min() >= 1, "empty slot: epilogue would be skipped"
        # chunk meta: (slot, k_in_slot, table) in processing order
        # b-table chunks first: their gathers only depend on the (smaller,
        # first-written) G1b table, overlapping the tail of stage A
        meta = []
        for s in range(NBLK):
            k = 0
            for _ in range(int(Kb[s])):
                meta.append((s, k, 1)); k += 1
            for _ in range(int(Ka[s])):
                meta.append((s, k, 0)); k += 1
        # gather ops: runs of <=GRP same-table consecutive chunks
        ops = []   # (chunk_start, n_chunks, table)
        i = 0
        while i < NCH:
            t = meta[i][2]
            j = i
            while j < NCH and j - i < GRP and meta[j][2] == t:
                j += 1
            ops.append((i, j - i, t))
            i = j
        NOPS = len(ops)

        EPAD = NCH * P
        SRCK = np.zeros((NCORES, EPAD), dtype=np.int64)
        DREL = np.full((NCORES, EPAD), 255.0, dtype=np.float32)
        for c in range(NCORES):
            base_node = c * NPC
            cb = 0
            for s in range(NBLK):
                blo = base_node + s * P
                bhi = min(blo + P, base_node + NPC)
                lo = np.searchsorted(dst, blo, side="left")
                hi = np.searchsorted(dst, bhi, side="left")
                sk = skey[lo:hi]
                dr = (dst[lo:hi] - blo).astype(np.float32)
                a_mask = sk < HALF
                for which, KK, pad in ((~a_mask, Kb[s], HALF),
                                       (a_mask, Ka[s], 0)):
                    cnt = int(which.sum())
                    pos = cb * P
                    SRCK[c, pos:pos + cnt] = sk[which]
                    # pad indices must stay valid for the table half
                    SRCK[c, pos + cnt:(cb + int(KK)) * P] = pad
                    DREL[c, pos:pos + cnt] = dr[which]
                    cb += int(KK)
        # static one-hot matrices, bf16:
        #   S8W[c, e, ch*128+d] = (DREL[ch, e] == d)   (segment-matmul lhsT)
        #   ST8W[c, d, ch*128+e] = same, transposed    (alpha-dst-expand lhsT)
        drel_ch = DREL.reshape(NCORES, NCH, P)
        oh = (drel_ch[:, :, :, None] ==
              np.arange(P, dtype=np.float32)[None, None, None, :])
        S8W = np.ascontiguousarray(
            oh.transpose(0, 2, 1, 3).reshape(NCORES, P, NCH * P)
        ).astype(ml_dtypes.float8_e4m3fn)
        ST8W = np.ascontiguousarray(
            oh.transpose(0, 3, 1, 2).reshape(NCORES, P, NCH * P)
        ).astype(ml_dtypes.float8_e4m3fn)
        # wrapped int16 indices per gather op, [128, NOPS*64]
        IDXW = np.zeros((NCORES, P, NOPS * GRP * 8), dtype=np.int16)
        for c in range(NCORES):
            for o, (c0, ncg, t) in enumerate(ops):
                iv = SRCK[c, c0 * P:(c0 + ncg) * P] - (HALF if t else 0)
                w = iv.reshape(-1, 16).T.astype(np.int16)  # [16, n/16]
                IDXW[c, :, o * GRP * 8: o * GRP * 8 + w.shape[1]] = \
                    np.tile(w, (8, 1))
        return dict(NCH=NCH, meta=meta, ops=ops, NOPS=NOPS,
                    Ktot=[int(Ka[s] + Kb[s]) for s in range(NBLK)],
                    S8W=S8W, ST8W=ST8W, IDXW=IDXW)

    L1 = build_layer(1)
    L2 = build_layer(2)

    # per-core own-node x slice (transposed, padded): alpha_dst + self loops
    xTself = np.zeros((NCORES, D, NROWC), dtype=np.float32)
    for c in range(NCORES):
        hi = min(c * NPC + NROWC, N)
        xTself[c, :, :hi - c * NPC] = x[c * NPC:hi].T

    xT = np.zeros((D, NROW1), dtype=np.float32)
    xT[:, :N] = x.T
    # rhs1 = [W1 | per-head W1@as1 (3) | per-head W1@ad1 (3)]  [128, 198]
    as_cols = np.stack([W1[:, h * HID:(h + 1) * HID] @ as1[h]
                        for h in range(H)], axis=1)
    ad_cols = np.stack([W1[:, h * HID:(h + 1) * HID] @ ad1[h]
                        for h in range(H)], axis=1)
    RHS1 = np.ascontiguousarray(
        np.concatenate([W1, as_cols, ad_cols], axis=1).astype(np.float32))
    # rhs2 = [W2 | W2@as2 | W2@ad2]  [192, 66]
    RHS2 = np.ascontiguousarray(np.concatenate(
        [W2, (W2 @ as2[0])[:, None], (W2 @ ad2[0])[:, None]],
        axis=1).astype(np.float32))

    shared = {
        "xT": xT,
        "RHS1": RHS1,
        "RHS2lo": np.ascontiguousarray(RHS2[:P]),
        "RHS2hi": np.ascontiguousarray(RHS2[P:]),
        "B1": np.ascontiguousarray(np.broadcast_to(b1, (P, F1))),
        "B2": np.ascontiguousarray(np.broadcast_to(b2, (P, F2))),
    }
    percore = []
    for c in range(NCORES):
        percore.append({
            "S8W1": L1["S8W"][c], "ST8W1": L1["ST8W"][c],
            "IDXW1": L1["IDXW"][c],
            "S8W2": L2["S8W"][c], "ST8W2": L2["ST8W"][c],
            "IDXW2": L2["IDXW"][c],
            "xTself": xTself[c],
        })
    key = (tuple(L1["Ktot"]), tuple(x[0] for x in L1["ops"]),
           tuple(x[1] for x in L1["ops"]), tuple(x[2] for x in L1["ops"]),
           tuple(L2["Ktot"]), tuple(x[0] for x in L2["ops"]),
           tuple(x[1] for x in L2["ops"]), tuple(x[2] for x in L2["ops"]))
    return key, (L1, L2), shared, percore


def _ap_view(ap, extra_offset, free_dims):
    import concourse.bass as bass

    return bass.AP(
        tensor=ap.tensor, offset=ap.offset + extra_offset,
        ap=[list(ap.ap[0])] + [list(d) for d in free_dims],
    )


def _dram_rows_3d(t, row0, nrows_inner, nblocks, width):
    """AP over DRAM tensor t rows [row0, row0+nblocks*nrows_inner) viewed as
    [nrows_inner (partition), nblocks, width]."""
    base = t[row0:row0 + nrows_inner, :]
    return _ap_view(base, 0,
                    [[nrows_inner * width, nblocks], [1, width]])


def _build(L1, L2):
    import concourse.bass as bass
    import concourse.bacc as bacc
    import concourse.tile as tile
    from concourse import mybir
    from concourse.masks import make_identity
    from concourse.library_config import mlp
    from contextlib import ExitStack

    f32 = mybir.dt.float32
    bf16 = mybir.dt.bfloat16
    i16 = mybir.dt.int16
    fp8 = mybir.dt.float8e4
    AT = mybir.ActivationFunctionType
    OP = mybir.AluOpType

    nc = bacc.Bacc("TRN2", target_bir_lowering=False, debug=False,
                   num_devices=NCORES, num_swdge_queues=4)

    xT = nc.dram_tensor("xT", [D, NROW1], f32, kind="ExternalInput")
    RHS1 = nc.dram_tensor("RHS1", [D, F1 + 6], f32, kind="ExternalInput")
    RHS2lo = nc.dram_tensor("RHS2lo", [P, F2 + 2], f32, kind="ExternalInput")
    RHS2hi = nc.dram_tensor("RHS2hi", [F1 - P, F2 + 2], f32,
                            kind="ExternalInput")
    B1 = nc.dram_tensor("B1", [P, F1], f32, kind="ExternalInput")
    B2 = nc.dram_tensor("B2", [P, F2], f32, kind="ExternalInput")
    S8W1 = nc.dram_tensor("S8W1", [P, L1["NCH"] * P], fp8,
                          kind="ExternalInput")
    ST8W1 = nc.dram_tensor("ST8W1", [P, L1["NCH"] * P], fp8,
                           kind="ExternalInput")
    IDXW1 = nc.dram_tensor("IDXW1", [P, L1["NOPS"] * GRP * 8], i16,
                           kind="ExternalInput")
    S8W2 = nc.dram_tensor("S8W2", [P, L2["NCH"] * P], fp8,
                          kind="ExternalInput")
    ST8W2 = nc.dram_tensor("ST8W2", [P, L2["NCH"] * P], fp8,
                           kind="ExternalInput")
    IDXW2 = nc.dram_tensor("IDXW2", [P, L2["NOPS"] * GRP * 8], i16,
                           kind="ExternalInput")
    XTS = nc.dram_tensor("xTself", [D, NROWC], f32, kind="ExternalInput")
    OUT = nc.dram_tensor("out", [NROWC, F2], f32, kind="ExternalOutput")

    G1a = nc.dram_tensor("G1a", [HALF, G1W], bf16, kind="Internal")
    G1b = nc.dram_tensor("G1b", [NROW1 - HALF, G1W], bf16, kind="Internal")
    G2L = nc.dram_tensor("G2L", [NROWC, G2W], bf16, kind="Internal")
    G2F = nc.dram_tensor("G2F", [NROWC * NCORES, G2W], bf16,
                         addr_space="Shared", kind="Internal")

    with tile.TileContext(nc) as tc, ExitStack() as ctx:
        consts = ctx.enter_context(tc.tile_pool(name="consts", bufs=1))
        sbA = ctx.enter_context(tc.tile_pool(name="sbA", bufs=3))
        psA = ctx.enter_context(tc.tile_pool(name="psA", bufs=3, space="PSUM"))
        psum = ctx.enter_context(tc.tile_pool(name="psum", bufs=2, space="PSUM"))
        psad = ctx.enter_context(tc.tile_pool(name="psad", bufs=1, space="PSUM"))
        pst = ctx.enter_context(tc.tile_pool(name="pst", bufs=2, space="PSUM"))
        gpool = ctx.enter_context(tc.tile_pool(name="gpool", bufs=6))
        fpool = ctx.enter_context(tc.tile_pool(name="fpool", bufs=6))
        spool = ctx.enter_context(tc.tile_pool(name="spool", bufs=6))
        ipool = ctx.enter_context(tc.tile_pool(name="ipool", bufs=6))
        epool = ctx.enter_context(tc.tile_pool(name="epool", bufs=4))

        nc.gpsimd.load_library(mlp)

        # ---------------- constants / weight prep ----------------
        b1t = consts.tile([P, F1], f32)
        nc.sync.dma_start(out=b1t[:], in_=B1[:])
        b2t = consts.tile([P, F2], f32)
        nc.sync.dma_start(out=b2t[:], in_=B2[:])
        ident = consts.tile([P, P], f32)
        make_identity(nc, ident[:])
        rhs1 = consts.tile([P, F1 + 6], f32)
        nc.sync.dma_start(out=rhs1[:], in_=RHS1[:])
        rhs2_lo = consts.tile([P, F2 + 2], f32)
        nc.sync.dma_start(out=rhs2_lo[:], in_=RHS2lo[:])
        rhs2_hi = consts.tile([F1 - P, F2 + 2], f32)
        nc.sync.dma_start(out=rhs2_hi[:], in_=RHS2hi[:])
        xtself = consts.tile([P, NROWC], f32)
        nc.sync.dma_start(out=xtself[:], in_=XTS[:])
        # layer-2 per-slot stashes (filled by epi1)
        g2self = consts.tile([P, NBLK, G2W], bf16)
        adb2_all = consts.tile([P, NBLK], f32)

        # ---------------- stage A (G1b tiles first) ----------------
        tiles = list(range(HALF // P, NT)) + list(range(HALF // P))
        groups = []
        i = 0
        while i < len(tiles):
            j = i
            while (j < len(tiles) and j - i < TBATCH
                   and tiles[j] == tiles[i] + (j - i)):
                j += 1
            groups.append((tiles[i], j - i))
            i = j
        for t0, nb in groups:
            xt4 = sbA.tile([P, TBATCH * P], f32, tag="xt")
            nc.sync.dma_start(out=xt4[:, :nb * P],
                              in_=xT[:, t0 * P:(t0 + nb) * P])
            gbf4 = sbA.tile([P, TBATCH, G1W], bf16, tag="gbf")
            for j in range(nb):
                pa = psA.tile([P, 200], f32, tag="mmA")
                nc.tensor.matmul(out=pa[:, :F1 + 3],
                                 lhsT=xt4[:, j * P:(j + 1) * P],
                                 rhs=rhs1[:, :F1 + 3], start=True, stop=True)
                nc.scalar.activation(out=gbf4[:, j, :F1], in_=pa[:, :F1],
                                     func=AT.Copy)
                gf32 = gbf4[:].bitcast(f32)
                nc.vector.tensor_copy(out=gf32[:, j, 96:99],
                                      in_=pa[:, F1:F1 + 3])
            if t0 >= HALF // P:
                dst_ap = _dram_rows_3d(G1b, (t0 - HALF // P) * P, P, nb, G1W)
            else:
                dst_ap = _dram_rows_3d(G1a, t0 * P, P, nb, G1W)
            nc.scalar.dma_start(out=dst_ap, in_=gbf4[:, :nb, :])

        # ---------------- generic edge phase ----------------
        def edge_layer(LM, TBLa, TBLb, width, nfeat, as_f32col, s8w, st8w,
                       idxwi, ps_width, nheads, new_slot, slot_epilogue):
            NCH = LM["NCH"]
            meta = LM["meta"]
            ops = LM["ops"]
            Ktot = LM["Ktot"]
            ps_cur = [None]
            adb_cur = [None]
            fw = nfeat + nheads  # F8 row width

            for o, (c0, ncg, tb) in enumerate(ops):
                idxt = ipool.tile([P, GRP * 8], i16, tag="idxt", name="idxt")
                nc.sync.dma_start(
                    out=idxt[:, :ncg * 8],
                    in_=idxwi[:, o * GRP * 8:o * GRP * 8 + ncg * 8])
                s8t = spool.tile([P, GRP * P], fp8, tag="s8", name="s8t")
                nc.sync.dma_start(
                    out=s8t[:, :ncg * P],
                    in_=s8w[:, c0 * P:(c0 + ncg) * P])
                st8 = spool.tile([P, GRP * P], fp8, tag="st8", name="st8")
                nc.scalar.dma_start(
                    out=st8[:, :ncg * P],
                    in_=st8w[:, c0 * P:(c0 + ncg) * P])
                grow = gpool.tile([P, GRP, width], bf16, tag="grow",
                                  name="grow")
                nidx = ncg * P
                nc.gpsimd.dma_gather(
                    grow[:, :ncg, :], (TBLb if tb else TBLa)[:],
                    idxt[:, :ncg * 8], nidx, nidx, width,
                    queue_num=o % 4)
                # alpha_dst expansion: adp[e, h] = ST8^T x adb
                adp = psad.tile([P, 36], f32, tag="adp", name="adp")
                for jj in range(ncg):
                    s, k, _tb2 = meta[c0 + jj]
                    if k == 0:
                        new_slot(s, ps_cur, adb_cur, adp)
                    nc.tensor.matmul(
                        out=adp[:, jj * nheads:(jj + 1) * nheads],
                        lhsT=st8[:, jj * P:(jj + 1) * P],
                        rhs=adb_cur[0][:, :nheads],
                        start=True, stop=True)
                # logits -> exp -> weighted features
                growf = grow[:].bitcast(f32)
                t8 = epool.tile([P, GRP * nheads], f32, tag="t8", name="t8")
                nc.vector.tensor_tensor(
                    out=_ap_view(t8[:], 0, [[nheads, ncg], [1, nheads]]),
                    in0=_ap_view(growf, as_f32col,
                                 [[width // 2, ncg], [1, nheads]]),
                    in1=_ap_view(adp[:], 0, [[nheads, ncg], [1, nheads]]),
                    op=OP.add)
                # exp(lrelu(t)) == max(exp(t), exp(SLOPE*t)) exactly
                e2 = epool.tile([P, GRP * nheads], f32, tag="r8", name="e2")
                nc.scalar.activation(out=e2[:, :ncg * nheads],
                                     in_=t8[:, :ncg * nheads],
                                     func=AT.Exp, scale=SLOPE)
                F8 = fpool.tile([P, GRP * fw], bf16, tag="f8", name="f8")
                nc.scalar.activation(
                    out=_ap_view(F8[:], nfeat, [[fw, ncg], [1, nheads]]),
                    in_=_ap_view(t8[:], 0, [[nheads, ncg], [1, nheads]]),
                    func=AT.Exp)
                nc.vector.tensor_tensor(
                    out=_ap_view(F8[:], nfeat, [[fw, ncg], [1, nheads]]),
                    in0=_ap_view(F8[:], nfeat, [[fw, ncg], [1, nheads]]),
                    in1=_ap_view(e2[:], 0, [[nheads, ncg], [1, nheads]]),
                    op=OP.max)
                hd = nfeat // nheads
                nc.vector.tensor_tensor(
                    out=_ap_view(F8[:], 0, [[fw, ncg], [hd, nheads], [1, hd]]),
                    in0=_ap_view(grow[:], 0,
                                 [[width, ncg], [hd, nheads], [1, hd]]),
                    in1=_ap_view(F8[:], nfeat,
                                 [[fw, ncg], [1, nheads], [0, hd]]),
                    op=OP.mult)
                for jj in range(ncg):
                    s, k, _tb2 = meta[c0 + jj]
                    nc.tensor.matmul(
                        out=ps_cur[0][:, :ps_width],
                        lhsT=s8t[:, jj * P:(jj + 1) * P],
                        rhs=F8[:, jj * fw:jj * fw + ps_width],
                        start=(k == 0), stop=(k == Ktot[s] - 1))
                    if k == Ktot[s] - 1:
                        slot_epilogue(s, ps_cur[0])

        # ---------------- layer 1 slot hooks ----------------
        def new_slot1(s, ps_cur, adb_cur, adp):
            nc.tensor.matmul(out=adp[:, 32:32 + H],
                             lhsT=xtself[:, s * P:(s + 1) * P],
                             rhs=rhs1[:, F1 + 3:F1 + 6], start=True, stop=True)
            adbh = epool.tile([P, 4], bf16, tag="adbh", name="adbh")
            nc.vector.tensor_copy(out=adbh[:, :H], in_=adp[:, 32:32 + H])
            adb_cur[0] = adbh
            ps_cur[0] = psum.tile([P, 200], f32, tag="mm", name="ps_slot")

        # L1 epilogue: self contribution + h -> transpose -> G2 rows + stash
        def epi1(s, ps):
            # self loop: xw_self (+as/ad) for the slot's own 128 nodes
            pw = psA.tile([P, 200], f32, tag="mmA", name="pw_self")
            nc.tensor.matmul(out=pw[:, :F1 + 6],
                             lhsT=xtself[:, s * P:(s + 1) * P],
                             rhs=rhs1[:], start=True, stop=True)
            aw = epool.tile([P, 6], f32, tag="aw", name="aw")
            nc.vector.tensor_copy(out=aw[:], in_=pw[:, F1:F1 + 6])
            ts = epool.tile([P, H], f32, tag="ts", name="ts")
            nc.vector.tensor_tensor(out=ts[:], in0=aw[:, 0:3],
                                    in1=aw[:, 3:6], op=OP.add)
            es1 = epool.tile([P, H], f32, tag="es1", name="es1")
            nc.scalar.activation(out=es1[:], in_=ts[:], func=AT.Exp,
                                 scale=SLOPE)
            es2 = epool.tile([P, H], f32, tag="es2", name="es2")
            nc.scalar.activation(out=es2[:], in_=ts[:], func=AT.Exp)
            esm = epool.tile([P, H], f32, tag="esm", name="esm")
            nc.vector.tensor_tensor(out=esm[:], in0=es1[:], in1=es2[:],
                                    op=OP.max)
            fs = epool.tile([P, F1 + H], f32, tag="fs", name="fs")
            nc.vector.tensor_tensor(
                out=_ap_view(fs[:], 0, [[HID, H], [1, HID]]),
                in0=_ap_view(pw[:, :F1], 0, [[HID, H], [1, HID]]),
                in1=_ap_view(esm[:], 0, [[1, H], [0, HID]]),
                op=OP.mult)
            nc.vector.tensor_copy(out=fs[:, F1:F1 + H], in_=esm[:])
            tot = epool.tile([P, F1 + H], f32, tag="tot", name="tot")
            nc.vector.tensor_tensor(out=tot[:], in0=ps[:, :F1 + H],
                                    in1=fs[:], op=OP.add)
            # normalize + bias + relu
            rc = epool.tile([P, H], f32, tag="rc", name="rc")
            nc.vector.tensor_scalar_add(out=rc[:], in0=tot[:, F1:F1 + H],
                                        scalar1=EPS)
            rc2 = epool.tile([P, H], f32, tag="rc2", name="rc2")
            nc.vector.reciprocal(out=rc2[:], in_=rc[:])
            hm = epool.tile([P, F1], f32, tag="hm", name="hm")
            nc.vector.tensor_tensor(
                out=_ap_view(hm[:], 0, [[HID, H], [1, HID]]),
                in0=_ap_view(tot[:, :F1], 0, [[HID, H], [1, HID]]),
                in1=_ap_view(rc2[:], 0, [[1, H], [0, HID]]),
                op=OP.mult)
            hb = epool.tile([P, F1], f32, tag="hb", name="hb")
            nc.vector.tensor_tensor(out=hb[:], in0=hm[:], in1=b1t[:], op=OP.add)
            hr = epool.tile([P, F1], f32, tag="hr", name="hr")
            nc.scalar.activation(out=hr[:], in_=hb[:], func=AT.Relu)
            pt1 = pst.tile([P, P], f32, tag="tr", name="pt1")
            nc.tensor.transpose(out=pt1[:], in_=hr[:, :P], identity=ident[:])
            pt2 = pst.tile([P, P], f32, tag="tr", name="pt2")
            nc.tensor.transpose(out=pt2[0:F1 - P, :], in_=hr[:, P:F1],
                                identity=ident[:])
            ht1 = epool.tile([P, P], f32, tag="ht1", name="ht1")
            nc.vector.tensor_copy(out=ht1[:], in_=pt1[:])
            ht2 = epool.tile([F1 - P, P], f32, tag="ht2", name="ht2")
            nc.vector.tensor_copy(out=ht2[:], in_=pt2[0:F1 - P, :])
            pg = psA.tile([P, 200], f32, tag="mmA", name="pg")
            nc.tensor.matmul(out=pg[:, :F2 + 2], lhsT=ht1[:], rhs=rhs2_lo[:],
                             start=True, stop=False)
            nc.tensor.matmul(out=pg[:, :F2 + 2], lhsT=ht2[:], rhs=rhs2_hi[:],
                             start=False, stop=True)
            g2 = epool.tile([P, G2W], bf16, tag="g2", name="g2")
            nc.vector.tensor_copy(out=g2[:, :F2], in_=pg[:, :F2])
            g2f = g2[:].bitcast(f32)
            nc.vector.tensor_copy(out=g2f[:, 32:33], in_=pg[:, F2:F2 + 1])
            nc.vector.tensor_copy(out=g2self[:, s, :], in_=g2[:])
            nc.vector.tensor_copy(out=adb2_all[:, s:s + 1],
                                  in_=pg[:, F2 + 1:F2 + 2])
            nc.sync.dma_start(out=G2L[s * P:(s + 1) * P, :], in_=g2[:])

        edge_layer(L1, G1a, G1b, G1W, F1, 96, S8W1, ST8W1, IDXW1,
                   F1 + H, H, new_slot1, epi1)

        # ---------------- AllGather ----------------
        nc.gpsimd.collective_compute(
            "AllGather", mybir.AluOpType.bypass,
            replica_groups=[list(range(NCORES))],
            ins=[G2L.ap().opt()], outs=[G2F.ap().opt()])

        # ---------------- layer 2 ----------------
        def new_slot2(s, ps_cur, adb_cur, adp):
            adbh = epool.tile([P, 4], bf16, tag="adbh", name="adbh2")
            nc.vector.tensor_copy(out=adbh[:, :1], in_=adb2_all[:, s:s + 1])
            adb_cur[0] = adbh
            ps_cur[0] = psum.tile([P, 200], f32, tag="mm", name="ps_slot2")

        def epi2(s, ps):
            g2sf = g2self[:].bitcast(f32)
            ts = epool.tile([P, 1], f32, tag="tsB", name="tsB")
            nc.vector.tensor_tensor(
                out=ts[:], in0=g2sf[:, s, 32:33],
                in1=adb2_all[:, s:s + 1], op=OP.add)
            es1 = epool.tile([P, 1], f32, tag="es1B", name="es1B")
            nc.scalar.activation(out=es1[:], in_=ts[:], func=AT.Exp,
                                 scale=SLOPE)
            es2 = epool.tile([P, 1], f32, tag="es2B", name="es2B")
            nc.scalar.activation(out=es2[:], in_=ts[:], func=AT.Exp)
            esm = epool.tile([P, 1], f32, tag="esmB", name="esmB")
            nc.vector.tensor_tensor(out=esm[:], in0=es1[:], in1=es2[:],
                                    op=OP.max)
            fs = epool.tile([P, F2 + 1], f32, tag="fsB", name="fsB")
            nc.vector.tensor_tensor(out=fs[:, :F2],
                                    in0=g2self[:, s, :F2],
                                    in1=esm[:].to_broadcast([P, F2]),
                                    op=OP.mult)
            nc.vector.tensor_copy(out=fs[:, F2:F2 + 1], in_=esm[:])
            tot = epool.tile([P, F2 + 1], f32, tag="totB", name="totB")
            nc.vector.tensor_tensor(out=tot[:], in0=ps[:, :F2 + 1],
                                    in1=fs[:], op=OP.add)
            rc = epool.tile([P, 1], f32, tag="rcB", name="rcB")
            nc.vector.tensor_scalar_add(out=rc[:], in0=tot[:, F2:F2 + 1],
                                        scalar1=EPS)
            rc2 = epool.tile([P, 1], f32, tag="rcB2", name="rcB2")
            nc.vector.reciprocal(out=rc2[:], in_=rc[:])
            om = epool.tile([P, F2], f32, tag="om", name="om")
            nc.vector.tensor_tensor(out=om[:], in0=tot[:, :F2],
                                    in1=rc2[:].to_broadcast([P, F2]),
                                    op=OP.mult)
            ob = epool.tile([P, F2], f32, tag="ob", name="ob")
            nc.vector.tensor_tensor(out=ob[:], in0=om[:], in1=b2t[:], op=OP.add)
            orl = epool.tile([P, F2], f32, tag="orl", name="orl")
            nc.scalar.activation(out=orl[:], in_=ob[:], func=AT.Relu)
            nc.sync.dma_start(out=OUT[s * P:(s + 1) * P, :], in_=orl[:])

        # G2F views for the two index halves (offsets stay < 2^24 bytes)
        g2fa = G2F[0:HALF, :]
        g2fb = G2F[HALF:NROWC * NCORES, :]
        edge_layer(L2, g2fa, g2fb, G2W, F2, 32, S8W2, ST8W2, IDXW2,
                   F2 + 1, 1, new_slot2, epi2)

    nc.compile()
    return nc


def _get_compiled(key, layers):
    if key not in _compiled:
        _compiled[key] = _build(layers[0], layers[1])
    return _compiled[key]


def run(inputs, **runkw):
    from concourse import bass_utils

    key, layers, shared, percore = _host_prep(inputs)
    nc = _get_compiled(key, layers)
    in_maps = []
    for c in range(NCORES):
        m = dict(shared)
        m.update(percore[c])
        in_maps.append(m)
    res = bass_utils.run_bass_kernel_spmd(
        nc, in_maps, core_ids=list(range(NCORES)), **runkw)
    return res


def assemble(results):
    out = np.empty((N, F2), dtype=np.float32)
    for c in range(NCORES):
        out[c * NPC:(c + 1) * NPC] = results[c]["out"][:NPC]
    return out


def kernel(**inputs):
    res = run(inputs)
    return assemble(res.results)
